# revision 14
# baseline (speedup 1.0000x reference)
"""EulerAttentionHead Trainium2 kernel.

Reference computation (B=4, S=4096, D=1024, H=128):
    Q = x @ Wq.T + bq ; K = x @ Wk.T + bk ; V = x @ Wv.T + bv
    theta_{q,k} = {Q,K} / (wavelengths + 1e-8) + phase_bias
    sim = cos(tq) @ cos(tk).T + sin(tq) @ sin(tk).T
    out = softmax(sim / sqrt(H)) @ V @ Wo.T + bo

Sharding: 8 cores = 4 batches x 2 query-halves. Each core handles one
batch's full key/value set (4096 keys) and 2048 queries. The host rolls
x so each core's query rows are rows 0:2048 of its input (softmax over
keys is permutation-invariant, so key order doesn't matter).

Per-core dataflow (layouts chosen so matmul contraction dims sit on
SBUF partitions; PE datapath in fp16 with fp32 PSUM accumulation):
  - x and the four weight matrices are cast to fp16 on the host and
    transposed on-chip by the DMA xbar (2-byte dtype), so the PE never
    spends cycles transposing inputs
  - Q.T/K.T/V.T = W.T-stationary fp16 matmuls over x.T
  - theta -> Cody-Waite range reduction (custom DVE ops) -> ACT Sin for
    sin and cos (Sin LUT domain is [-pi, pi])
  - S.T tile [k,q] = Fk-stationary fp16 matmul; exp(S/sqrt(H) - 1) via
    ACT -> E.T fp16 (the -1 keeps exp under fp16 max; it cancels in the
    softmax normalization)
  - AV: lhsT = E.T (fp16, FWL), rhs = [V | ones] so the softmax
    denominator comes out as PSUM column 128 of O for free
  - normalize O rows by 1/denominator during PSUM eviction (ACT scale),
    PE-transpose O, project with Wo.T (fp16), add bo, DMA out.
"""

import math

import numpy as np

import concourse.mybir as mybir
import concourse.tile as tile
from concourse import bacc
from concourse.masks import make_identity

F32 = mybir.dt.float32
F16 = mybir.dt.float16
AF = mybir.ActivationFunctionType

B, S, D, H = 4, 4096, 1024, 128
SQ = S // 2  # queries per core
N_CORES = 8

TWO_PI = 2.0 * math.pi
INV_TWO_PI = 1.0 / TWO_PI
MAGIC = 12582912.0  # 1.5 * 2**23: fp32 (u + M) - M == round(u)
PI_CLAMP = 3.1415925  # just inside fp32 pi; ACT Sin domain is [-pi, pi]
INV_SQRT_H = 1.0 / math.sqrt(H)


def _cody_waite_consts():
    # Split 2*pi into c1 + c2 + c3, c1/c2 with zeroed low mantissa bits so
    # theta - k*c1 - k*c2 - k*c3 cancels exactly for small integer k.
    def chop(v):
        f = np.float32(v)
        i = f.view(np.uint32) & np.uint32(0xFFFFF000)
        return float(i.view(np.float32))

    c1 = chop(TWO_PI)
    c2 = chop(TWO_PI - c1)
    c3 = float(np.float32(TWO_PI - c1 - c2))
    return c1, c2, c3


C1, C2, C3 = _cody_waite_consts()

_CACHED = None


def _build():
    nc = bacc.Bacc("TRN2", target_bir_lowering=False, debug=False,
                   num_devices=N_CORES)

    x16 = nc.dram_tensor("x16", (S, D), F16, kind="ExternalInput")
    Wq16 = nc.dram_tensor("Wq16", (H, D), F16, kind="ExternalInput")
    Wk16 = nc.dram_tensor("Wk16", (H, D), F16, kind="ExternalInput")
    Wv16 = nc.dram_tensor("Wv16", (H, D), F16, kind="ExternalInput")
    Wo16 = nc.dram_tensor("Wo16", (D, H), F16, kind="ExternalInput")
    bq = nc.dram_tensor("bq", (H, 1), F32, kind="ExternalInput")
    bk = nc.dram_tensor("bk", (H, 1), F32, kind="ExternalInput")
    bv = nc.dram_tensor("bv", (H, 1), F32, kind="ExternalInput")
    bo = nc.dram_tensor("bo", (1, D), F32, kind="ExternalInput")
    wav = nc.dram_tensor("wavelengths", (H, 1), F32, kind="ExternalInput")
    phase = nc.dram_tensor("phase_bias", (H, 1), F32, kind="ExternalInput")
    y = nc.dram_tensor("y", (SQ, D), F32, kind="ExternalOutput")

    with tile.TileContext(nc) as tc, \
            tc.tile_pool(name="const", bufs=1) as const, \
            tc.tile_pool(name="big", bufs=1) as big, \
            tc.tile_pool(name="xa", bufs=2) as xa_pool, \
            tc.tile_pool(name="tmp", bufs=2) as tmp, \
            tc.tile_pool(name="psum_t", bufs=1, space="PSUM") as psum_t, \
            tc.tile_pool(name="psum_mm", bufs=2, space="PSUM") as psum_mm, \
            tc.tile_pool(name="psum_o", bufs=4, space="PSUM") as psum_o:

        # ---- constants -------------------------------------------------
        ident_h = const.tile([128, 128], F16)
        make_identity(nc, ident_h)

        wav_sb = const.tile([H, 1], F32)
        nc.scalar.dma_start(wav_sb, wav.ap())
        phase_sb = const.tile([H, 1], F32)
        nc.scalar.dma_start(phase_sb, phase.ap())
        bq_sb = const.tile([H, 1], F32)
        nc.scalar.dma_start(bq_sb, bq.ap())
        bk_sb = const.tile([H, 1], F32)
        nc.scalar.dma_start(bk_sb, bk.ap())
        bv_sb = const.tile([H, 1], F32)
        nc.scalar.dma_start(bv_sb, bv.ap())

        inv_w = const.tile([H, 1], F32)
        tw = const.tile([H, 1], F32)
        nc.vector.tensor_scalar(tw, wav_sb, 1e-8, None, mybir.AluOpType.add)
        nc.vector.reciprocal(inv_w, tw)
        cadd_q = const.tile([H, 1], F32)
        nc.vector.tensor_scalar(cadd_q, bq_sb, inv_w, phase_sb,
                                mybir.AluOpType.mult, mybir.AluOpType.add)
        cadd_k = const.tile([H, 1], F32)
        nc.vector.tensor_scalar(cadd_k, bk_sb, inv_w, phase_sb,
                                mybir.AluOpType.mult, mybir.AluOpType.add)
        inv_w2 = const.tile([H, 1], F32)
        nc.vector.tensor_scalar(inv_w2, inv_w, INV_TWO_PI, None,
                                mybir.AluOpType.mult)
        cadd_q2 = const.tile([H, 1], F32)
        nc.vector.tensor_scalar(cadd_q2, cadd_q, INV_TWO_PI, None,
                                mybir.AluOpType.mult)
        cadd_k2 = const.tile([H, 1], F32)
        nc.vector.tensor_scalar(cadd_k2, cadd_k, INV_TWO_PI, None,
                                mybir.AluOpType.mult)

        neg1 = const.tile([128, 1], F32)
        nc.vector.memset(neg1, -1.0)

        bo_row = const.tile([1, D], F32)
        nc.scalar.dma_start(bo_row, bo.ap())
        bo_tile = const.tile([128, D], F32)
        nc.gpsimd.partition_broadcast(bo_tile, bo_row)

        # ---- weight transposes (DMA xbar) -----------------------------
        WqT = const.tile([128, 8, 128], F16)
        nc.sync.dma_start_transpose(WqT, Wq16.ap())
        WkT = const.tile([128, 8, 128], F16)
        nc.scalar.dma_start_transpose(WkT, Wk16.ap())
        WvT = const.tile([128, 8, 128], F16)
        nc.sync.dma_start_transpose(WvT, Wv16.ap())
        WoT = const.tile([128, D], F16)  # [h, d]
        nc.scalar.dma_start_transpose(WoT, Wo16.ap())

        # ---- persistent activations -----------------------------------
        Fq_cos = big.tile([128, SQ], F16)
        Fq_sin = big.tile([128, SQ], F16)
        Fk_cos = big.tile([128, S], F16)
        Fk_sin = big.tile([128, S], F16)
        Vn = big.tile([128, 32, 129], F16)  # [k_part, k_tile, h | ones]
        nc.vector.memset(Vn[:, :, 128:129], 1.0)
        osb = big.tile([128, 16, 129], F32)  # raw [O | denom] per q-subtile

        # ---- phase A: x.T (DMA), projections, sin/cos, V --------------
        def theta_path(pp, cadd, cadd2, cos_slice, sin_slice):
            th = tmp.tile([128, 512], F32, tag="th")
            nc.scalar.activation(th, pp, AF.Identity, bias=cadd, scale=inv_w)
            u = tmp.tile([128, 512], F32, tag="u")
            nc.scalar.activation(u, pp, AF.Identity, bias=cadd2,
                                 scale=inv_w2)
            kk = tmp.tile([128, 512], F32, tag="kk")
            nc.gpsimd.tensor_scalar(kk, u, MAGIC, MAGIC,
                                    mybir.AluOpType.add,
                                    mybir.AluOpType.subtract)
            thr = tmp.tile([128, 512], F32, tag="thr")
            nc.vector.cody_waite_cascade(thr, th, kk, C1, C2, C3)
            nc.vector.tensor_scalar(thr, thr, PI_CLAMP, -PI_CLAMP,
                                    mybir.AluOpType.min, mybir.AluOpType.max)
            nc.scalar.activation(sin_slice, thr, AF.Sin)
            thc = tmp.tile([128, 512], F32, tag="thc")
            nc.vector.add_range_wrap(thc, thr, math.pi / 2, math.pi, TWO_PI)
            nc.scalar.activation(cos_slice, thc, AF.Sin)

        for sc in range(8):
            xt = xa_pool.tile([128, 8, 512], F16, tag="xt", bufs=4)
            eng = nc.sync if sc % 2 == 0 else nc.scalar
            eng.dma_start_transpose(
                xt, x16.ap()[sc * 512:(sc + 1) * 512, :])

            def proj(wt):
                pp = psum_mm.tile([128, 512], F32, tag="mm512")
                for dc in range(8):
                    nc.tensor.matmul(pp, wt[:, dc, :], xt[:, dc, :],
                                     start=(dc == 0), stop=(dc == 7))
                return pp

            sl = slice(sc * 512, (sc + 1) * 512)
            theta_path(proj(WkT), cadd_k, cadd_k2,
                       Fk_cos[:, sl], Fk_sin[:, sl])

            ppv = proj(WvT)
            v16 = tmp.tile([128, 512], F16, tag="v16")
            nc.scalar.activation(v16, ppv, AF.Identity, bias=bv_sb)
            pv = psum_t.tile([128, 512], F16, tag="pt")
            for a in range(4):
                nc.tensor.transpose(pv[:, a * 128:(a + 1) * 128],
                                    v16[:, a * 128:(a + 1) * 128], ident_h)
            nc.vector.tensor_copy(
                Vn[:, sc * 4:(sc + 1) * 4, 0:128],
                pv.rearrange("p (a h) -> p a h", a=4))

            if sc < 4:
                theta_path(proj(WqT), cadd_q, cadd_q2,
                           Fq_cos[:, sl], Fq_sin[:, sl])

        # ---- phase B: attention per 512-query chunk -------------------
        for qc in range(4):
            qsl = slice(qc * 512, (qc + 1) * 512)
            opsums = [psum_o.tile([128, 129], F32, tag="opsum",
                                  name=f"opsum_{qc}_{i}")
                      for i in range(4)]
            for kt in range(32):
                st = psum_mm.tile([128, 512], F32, tag="mm512")
                ksl = slice(kt * 128, (kt + 1) * 128)
                nc.tensor.matmul(st, Fk_cos[:, ksl], Fq_cos[:, qsl],
                                 start=True, stop=False)
                nc.tensor.matmul(st, Fk_sin[:, ksl], Fq_sin[:, qsl],
                                 start=False, stop=True)
                et = tmp.tile([128, 512], F16, tag="et", bufs=3)
                nc.scalar.activation(et, st, AF.Exp, bias=neg1,
                                     scale=INV_SQRT_H)
                for qs in range(4):
                    nc.tensor.matmul(opsums[qs],
                                     et[:, qs * 128:(qs + 1) * 128],
                                     Vn[:, kt, :],
                                     start=(kt == 0), stop=(kt == 31),
                                     skip_group_check=True)
            for qs in range(4):
                nc.vector.tensor_copy(osb[:, qc * 4 + qs, :], opsums[qs])

        # ---- phase C: normalize + output projection -------------------
        for qc in range(4):
            otp = psum_t.tile([128, 512], F16, tag="pt")
            for qs in range(4):
                i = qc * 4 + qs
                rec = tmp.tile([128, 1], F32, tag="rec")
                nc.vector.reciprocal(rec, osb[:, i, 128:129])
                onrm = tmp.tile([128, 128], F16, tag="onrm")
                nc.scalar.activation(onrm, osb[:, i, 0:128], AF.Copy,
                                     scale=rec)
                nc.tensor.transpose(otp[:, qs * 128:(qs + 1) * 128],
                                    onrm, ident_h)
            ot = tmp.tile([128, 512], F16, tag="ot")
            nc.vector.tensor_copy(ot, otp)
            for qs in range(4):
                row = (qc * 4 + qs) * 128
                for half in range(2):
                    yp = psum_t.tile([128, 512], F32, tag="yp")
                    nc.tensor.matmul(yp, ot[:, qs * 128:(qs + 1) * 128],
                                     WoT[:, half * 512:(half + 1) * 512],
                                     start=True, stop=True)
                    ysb = tmp.tile([128, 512], F32, tag="ysb", bufs=3)
                    nc.vector.tensor_add(
                        ysb, yp, bo_tile[:, half * 512:(half + 1) * 512])
                    nc.gpsimd.dma_start(
                        y.ap()[row:row + 128,
                               half * 512:(half + 1) * 512], ysb)

    nc.compile()
    return nc


def get_nc():
    global _CACHED
    if _CACHED is None:
        _CACHED = _build()
    return _CACHED


def _in_maps(inputs):
    x = np.asarray(inputs["x"], np.float32)
    small = {
        "Wq16": np.asarray(inputs["Wq"], np.float16),
        "Wk16": np.asarray(inputs["Wk"], np.float16),
        "Wv16": np.asarray(inputs["Wv"], np.float16),
        "Wo16": np.asarray(inputs["Wo"], np.float16),
        "bq": np.asarray(inputs["bq"], np.float32).reshape(H, 1),
        "bk": np.asarray(inputs["bk"], np.float32).reshape(H, 1),
        "bv": np.asarray(inputs["bv"], np.float32).reshape(H, 1),
        "bo": np.asarray(inputs["bo"], np.float32).reshape(1, D),
        "wavelengths":
            np.asarray(inputs["wavelengths"], np.float32).reshape(H, 1),
        "phase_bias":
            np.asarray(inputs["phase_bias"], np.float32).reshape(H, 1),
    }
    maps = []
    for c in range(N_CORES):
        b, qoff = c // 2, (c % 2) * SQ
        xc = np.roll(x[b], -qoff, axis=0) if qoff else x[b]
        maps.append({"x16": np.ascontiguousarray(xc.astype(np.float16)),
                     **small})
    return maps


def kernel(**inputs):
    from concourse.bass_utils import run_bass_kernel_spmd

    nc = get_nc()
    res = run_bass_kernel_spmd(nc, _in_maps(inputs),
                               core_ids=list(range(N_CORES)))
    out = np.empty((B, S, D), np.float32)
    for c in range(N_CORES):
        b, qoff = c // 2, (c % 2) * SQ
        out[b, qoff:qoff + SQ] = res.results[c]["y"]
    return out


# revision 15
# speedup vs baseline: 1.3915x; 1.3915x over previous
"""EulerAttentionHead Trainium2 kernel.

Reference computation (B=4, S=4096, D=1024, H=128):
    Q = x @ Wq.T + bq ; K = x @ Wk.T + bk ; V = x @ Wv.T + bv
    theta_{q,k} = {Q,K} / (wavelengths + 1e-8) + phase_bias
    sim = cos(tq) @ cos(tk).T + sin(tq) @ sin(tk).T
    out = softmax(sim / sqrt(H)) @ V @ Wo.T + bo

Sharding: 8 cores = 4 batches x 2 query-halves. Each core handles one
batch's full key/value set (4096 keys) and 2048 queries. The host rolls
x so each core's query rows are rows 0:2048 of its input (softmax over
keys is permutation-invariant, so key order doesn't matter).

Per-core dataflow (layouts chosen so matmul contraction dims sit on
SBUF partitions; PE datapath in fp16 with fp32 PSUM accumulation):
  - x and the four weight matrices are cast to fp16 on the host and
    transposed on-chip by the DMA xbar (2-byte dtype), so the PE never
    spends cycles transposing inputs
  - Q.T/K.T/V.T = W.T-stationary fp16 matmuls over x.T
  - theta -> Cody-Waite range reduction (custom DVE ops) -> ACT Sin for
    sin and cos (Sin LUT domain is [-pi, pi])
  - S.T tile [k,q] = Fk-stationary fp16 matmul; exp(S/sqrt(H) - 1) via
    ACT -> E.T fp16 (the -1 keeps exp under fp16 max; it cancels in the
    softmax normalization)
  - AV: lhsT = E.T (fp16, FWL), rhs = [V | ones] so the softmax
    denominator comes out as PSUM column 128 of O for free
  - normalize O rows by 1/denominator during PSUM eviction (ACT scale),
    PE-transpose O, project with Wo.T (fp16), add bo, DMA out.
"""

import math

import numpy as np

import concourse.mybir as mybir
import concourse.tile as tile
from concourse import bacc
from concourse.masks import make_identity

F32 = mybir.dt.float32
F16 = mybir.dt.float16
AF = mybir.ActivationFunctionType

B, S, D, H = 4, 4096, 1024, 128
SQ = S // 2  # queries per core
N_CORES = 8

TWO_PI = 2.0 * math.pi
INV_TWO_PI = 1.0 / TWO_PI
MAGIC = 12582912.0  # 1.5 * 2**23: fp32 (u + M) - M == round(u)
PI_CLAMP = 3.1415925  # just inside fp32 pi; ACT Sin domain is [-pi, pi]
INV_SQRT_H = 1.0 / math.sqrt(H)


def _cody_waite_consts():
    # Split 2*pi into c1 + c2 + c3, c1/c2 with zeroed low mantissa bits so
    # theta - k*c1 - k*c2 - k*c3 cancels exactly for small integer k.
    def chop(v):
        f = np.float32(v)
        i = f.view(np.uint32) & np.uint32(0xFFFFF000)
        return float(i.view(np.float32))

    c1 = chop(TWO_PI)
    c2 = chop(TWO_PI - c1)
    c3 = float(np.float32(TWO_PI - c1 - c2))
    return c1, c2, c3


C1, C2, C3 = _cody_waite_consts()

_CACHED = None


def _build():
    nc = bacc.Bacc("TRN2", target_bir_lowering=False, debug=False,
                   num_devices=N_CORES)

    x16 = nc.dram_tensor("x16", (S, D), F16, kind="ExternalInput")
    Wq16 = nc.dram_tensor("Wq16", (H, D), F16, kind="ExternalInput")
    Wk16 = nc.dram_tensor("Wk16", (H, D), F16, kind="ExternalInput")
    Wv16 = nc.dram_tensor("Wv16", (H, D), F16, kind="ExternalInput")
    Wo16 = nc.dram_tensor("Wo16", (D, H), F16, kind="ExternalInput")
    bq = nc.dram_tensor("bq", (H, 1), F32, kind="ExternalInput")
    bk = nc.dram_tensor("bk", (H, 1), F32, kind="ExternalInput")
    bv = nc.dram_tensor("bv", (H, 1), F32, kind="ExternalInput")
    bo = nc.dram_tensor("bo", (1, D), F32, kind="ExternalInput")
    wav = nc.dram_tensor("wavelengths", (H, 1), F32, kind="ExternalInput")
    phase = nc.dram_tensor("phase_bias", (H, 1), F32, kind="ExternalInput")
    y = nc.dram_tensor("y", (SQ, D), F32, kind="ExternalOutput")

    with tile.TileContext(nc) as tc, \
            tc.tile_pool(name="const", bufs=1) as const, \
            tc.tile_pool(name="big", bufs=1) as big, \
            tc.tile_pool(name="xa", bufs=2) as xa_pool, \
            tc.tile_pool(name="tmp", bufs=2) as tmp, \
            tc.tile_pool(name="psum_t", bufs=1, space="PSUM") as psum_t, \
            tc.tile_pool(name="psum_mm", bufs=2, space="PSUM") as psum_mm, \
            tc.tile_pool(name="psum_o", bufs=4, space="PSUM") as psum_o:

        # ---- constants -------------------------------------------------
        ident_h = const.tile([128, 128], F16)
        make_identity(nc, ident_h)

        wav_sb = const.tile([H, 1], F32)
        nc.gpsimd.dma_start(wav_sb, wav.ap())
        phase_sb = const.tile([H, 1], F32)
        nc.gpsimd.dma_start(phase_sb, phase.ap())
        bq_sb = const.tile([H, 1], F32)
        nc.gpsimd.dma_start(bq_sb, bq.ap())
        bk_sb = const.tile([H, 1], F32)
        nc.gpsimd.dma_start(bk_sb, bk.ap())
        bv_sb = const.tile([H, 1], F32)
        nc.gpsimd.dma_start(bv_sb, bv.ap())

        inv_w = const.tile([H, 1], F32)
        tw = const.tile([H, 1], F32)
        nc.vector.tensor_scalar(tw, wav_sb, 1e-8, None, mybir.AluOpType.add)
        nc.vector.reciprocal(inv_w, tw)
        cadd_q = const.tile([H, 1], F32)
        nc.vector.tensor_scalar(cadd_q, bq_sb, inv_w, phase_sb,
                                mybir.AluOpType.mult, mybir.AluOpType.add)
        cadd_k = const.tile([H, 1], F32)
        nc.vector.tensor_scalar(cadd_k, bk_sb, inv_w, phase_sb,
                                mybir.AluOpType.mult, mybir.AluOpType.add)
        inv_w2 = const.tile([H, 1], F32)
        nc.vector.tensor_scalar(inv_w2, inv_w, INV_TWO_PI, None,
                                mybir.AluOpType.mult)
        cadd_q2 = const.tile([H, 1], F32)
        nc.vector.tensor_scalar(cadd_q2, cadd_q, INV_TWO_PI, None,
                                mybir.AluOpType.mult)
        cadd_k2 = const.tile([H, 1], F32)
        nc.vector.tensor_scalar(cadd_k2, cadd_k, INV_TWO_PI, None,
                                mybir.AluOpType.mult)

        neg1 = const.tile([128, 1], F32)
        nc.vector.memset(neg1, -1.0)

        bo_row = const.tile([1, D], F32)
        nc.gpsimd.dma_start(bo_row, bo.ap())
        bo_tile = const.tile([128, D], F32)
        nc.gpsimd.partition_broadcast(bo_tile, bo_row)

        # ---- weight transposes (DMA xbar) -----------------------------
        WqT = const.tile([128, 8, 128], F16)
        nc.sync.dma_start_transpose(WqT, Wq16.ap())
        WkT = const.tile([128, 8, 128], F16)
        nc.sync.dma_start_transpose(WkT, Wk16.ap())
        WvT = const.tile([128, 8, 128], F16)
        nc.sync.dma_start_transpose(WvT, Wv16.ap())
        WoT = const.tile([128, D], F16)  # [h, d]
        nc.sync.dma_start_transpose(WoT, Wo16.ap())

        # ---- persistent activations -----------------------------------
        Fq_cos = big.tile([128, SQ], F16)
        Fq_sin = big.tile([128, SQ], F16)
        Fk_cos = big.tile([128, S], F16)
        Fk_sin = big.tile([128, S], F16)
        Vn = big.tile([128, 32, 129], F16)  # [k_part, k_tile, h | ones]
        nc.vector.memset(Vn[:, :, 128:129], 1.0)
        osb = big.tile([128, 16, 129], F32)  # raw [O | denom] per q-subtile

        # ---- phase A: x.T (DMA), projections, sin/cos, V --------------
        def theta_path(pp, cadd, cadd2, cos_slice, sin_slice):
            th = tmp.tile([128, 512], F32, tag="th")
            nc.vector.tensor_scalar(th, pp, inv_w, cadd,
                                    mybir.AluOpType.mult, mybir.AluOpType.add)
            u = tmp.tile([128, 512], F32, tag="u")
            nc.vector.tensor_scalar(u, pp, inv_w2, cadd2,
                                    mybir.AluOpType.mult, mybir.AluOpType.add)
            kk = tmp.tile([128, 512], F32, tag="kk")
            nc.vector.tensor_scalar(kk, u, MAGIC, MAGIC,
                                    mybir.AluOpType.add,
                                    mybir.AluOpType.subtract)
            thr = tmp.tile([128, 512], F32, tag="thr")
            nc.vector.cody_waite_cascade(thr, th, kk, C1, C2, C3)
            nc.vector.tensor_scalar(thr, thr, PI_CLAMP, -PI_CLAMP,
                                    mybir.AluOpType.min, mybir.AluOpType.max)
            nc.scalar.activation(sin_slice, thr, AF.Sin)
            thc = tmp.tile([128, 512], F32, tag="thc")
            nc.vector.add_range_wrap(thc, thr, math.pi / 2, math.pi, TWO_PI)
            nc.scalar.activation(cos_slice, thc, AF.Sin)

        for sc in range(8):
            xt = xa_pool.tile([128, 8, 512], F16, tag="xt", bufs=4)
            nc.sync.dma_start_transpose(
                xt, x16.ap()[sc * 512:(sc + 1) * 512, :])

            def proj(wt):
                pp = psum_mm.tile([128, 512], F32, tag="mm512")
                for dc in range(8):
                    nc.tensor.matmul(pp, wt[:, dc, :], xt[:, dc, :],
                                     start=(dc == 0), stop=(dc == 7))
                return pp

            sl = slice(sc * 512, (sc + 1) * 512)
            theta_path(proj(WkT), cadd_k, cadd_k2,
                       Fk_cos[:, sl], Fk_sin[:, sl])

            ppv = proj(WvT)
            v16 = tmp.tile([128, 512], F16, tag="v16")
            nc.vector.tensor_scalar(v16, ppv, bv_sb, None, mybir.AluOpType.add)
            pv = psum_t.tile([128, 512], F16, tag="pt")
            for a in range(4):
                nc.tensor.transpose(pv[:, a * 128:(a + 1) * 128],
                                    v16[:, a * 128:(a + 1) * 128], ident_h)
            nc.vector.tensor_copy(
                Vn[:, sc * 4:(sc + 1) * 4, 0:128],
                pv.rearrange("p (a h) -> p a h", a=4))

            if sc < 4:
                theta_path(proj(WqT), cadd_q, cadd_q2,
                           Fq_cos[:, sl], Fq_sin[:, sl])

        # ---- phase B: attention per 512-query chunk -------------------
        for qc in range(4):
            qsl = slice(qc * 512, (qc + 1) * 512)
            opsums = [psum_o.tile([128, 129], F32, tag="opsum",
                                  name=f"opsum_{qc}_{i}")
                      for i in range(4)]
            for kt in range(32):
                st = psum_mm.tile([128, 512], F32, tag="mm512")
                ksl = slice(kt * 128, (kt + 1) * 128)
                nc.tensor.matmul(st, Fk_cos[:, ksl], Fq_cos[:, qsl],
                                 start=True, stop=False)
                nc.tensor.matmul(st, Fk_sin[:, ksl], Fq_sin[:, qsl],
                                 start=False, stop=True)
                et = tmp.tile([128, 512], F16, tag="et", bufs=3)
                nc.scalar.activation(et, st, AF.Exp, bias=neg1,
                                     scale=INV_SQRT_H)
                for qs in range(4):
                    nc.tensor.matmul(opsums[qs],
                                     et[:, qs * 128:(qs + 1) * 128],
                                     Vn[:, kt, :],
                                     start=(kt == 0), stop=(kt == 31),
                                     skip_group_check=True)
            for qs in range(4):
                nc.vector.tensor_copy(osb[:, qc * 4 + qs, :], opsums[qs])

        # ---- phase C: normalize + output projection -------------------
        for qc in range(4):
            otp = psum_t.tile([128, 512], F16, tag="pt")
            for qs in range(4):
                i = qc * 4 + qs
                rec = tmp.tile([128, 1], F32, tag="rec")
                nc.vector.reciprocal(rec, osb[:, i, 128:129])
                onrm = tmp.tile([128, 128], F16, tag="onrm")
                nc.scalar.activation(onrm, osb[:, i, 0:128], AF.Copy,
                                     scale=rec)
                nc.tensor.transpose(otp[:, qs * 128:(qs + 1) * 128],
                                    onrm, ident_h)
            ot = tmp.tile([128, 512], F16, tag="ot")
            nc.vector.tensor_copy(ot, otp)
            for qs in range(4):
                row = (qc * 4 + qs) * 128
                for half in range(2):
                    yp = psum_t.tile([128, 512], F32, tag="yp")
                    nc.tensor.matmul(yp, ot[:, qs * 128:(qs + 1) * 128],
                                     WoT[:, half * 512:(half + 1) * 512],
                                     start=True, stop=True)
                    ysb = tmp.tile([128, 512], F32, tag="ysb", bufs=3)
                    nc.vector.tensor_add(
                        ysb, yp, bo_tile[:, half * 512:(half + 1) * 512])
                    nc.gpsimd.dma_start(
                        y.ap()[row:row + 128,
                               half * 512:(half + 1) * 512], ysb)

    nc.compile()
    return nc


def get_nc():
    global _CACHED
    if _CACHED is None:
        _CACHED = _build()
    return _CACHED


def _in_maps(inputs):
    x = np.asarray(inputs["x"], np.float32)
    small = {
        "Wq16": np.asarray(inputs["Wq"], np.float16),
        "Wk16": np.asarray(inputs["Wk"], np.float16),
        "Wv16": np.asarray(inputs["Wv"], np.float16),
        "Wo16": np.asarray(inputs["Wo"], np.float16),
        "bq": np.asarray(inputs["bq"], np.float32).reshape(H, 1),
        "bk": np.asarray(inputs["bk"], np.float32).reshape(H, 1),
        "bv": np.asarray(inputs["bv"], np.float32).reshape(H, 1),
        "bo": np.asarray(inputs["bo"], np.float32).reshape(1, D),
        "wavelengths":
            np.asarray(inputs["wavelengths"], np.float32).reshape(H, 1),
        "phase_bias":
            np.asarray(inputs["phase_bias"], np.float32).reshape(H, 1),
    }
    maps = []
    for c in range(N_CORES):
        b, qoff = c // 2, (c % 2) * SQ
        xc = np.roll(x[b], -qoff, axis=0) if qoff else x[b]
        maps.append({"x16": np.ascontiguousarray(xc.astype(np.float16)),
                     **small})
    return maps


def kernel(**inputs):
    from concourse.bass_utils import run_bass_kernel_spmd

    nc = get_nc()
    res = run_bass_kernel_spmd(nc, _in_maps(inputs),
                               core_ids=list(range(N_CORES)))
    out = np.empty((B, S, D), np.float32)
    for c in range(N_CORES):
        b, qoff = c // 2, (c % 2) * SQ
        out[b, qoff:qoff + SQ] = res.results[c]["y"]
    return out


# revision 17
# speedup vs baseline: 1.4237x; 1.0231x over previous
"""EulerAttentionHead Trainium2 kernel.

Reference computation (B=4, S=4096, D=1024, H=128):
    Q = x @ Wq.T + bq ; K = x @ Wk.T + bk ; V = x @ Wv.T + bv
    theta_{q,k} = {Q,K} / (wavelengths + 1e-8) + phase_bias
    sim = cos(tq) @ cos(tk).T + sin(tq) @ sin(tk).T
    out = softmax(sim / sqrt(H)) @ V @ Wo.T + bo

Sharding: 8 cores = 4 batches x 2 query-halves. Each core handles one
batch's full key/value set (4096 keys) and 2048 queries. The host rolls
x so each core's query rows are rows 0:2048 of its input (softmax over
keys is permutation-invariant, so key order doesn't matter).

Per-core dataflow (layouts chosen so matmul contraction dims sit on
SBUF partitions; PE datapath in fp16 with fp32 PSUM accumulation):
  - x and the four weight matrices are cast to fp16 on the host and
    transposed on-chip by the DMA xbar (2-byte dtype), so the PE never
    spends cycles transposing inputs
  - Q.T/K.T/V.T = W.T-stationary fp16 matmuls over x.T
  - theta -> Cody-Waite range reduction (custom DVE ops) -> ACT Sin for
    sin and cos (Sin LUT domain is [-pi, pi])
  - S.T tile [k,q] = Fk-stationary fp16 matmul; exp(S/sqrt(H) - 1) via
    ACT -> E.T fp16 (the -1 keeps exp under fp16 max; it cancels in the
    softmax normalization)
  - AV: lhsT = E.T (fp16, FWL), rhs = [V | ones] so the softmax
    denominator comes out as PSUM column 128 of O for free
  - normalize O rows by 1/denominator during PSUM eviction (ACT scale),
    PE-transpose O, project with Wo.T (fp16), add bo, DMA out.
"""

import math

import numpy as np

import concourse.mybir as mybir
import concourse.tile as tile
from concourse import bacc
from concourse.masks import make_identity

F32 = mybir.dt.float32
F16 = mybir.dt.float16
AF = mybir.ActivationFunctionType

B, S, D, H = 4, 4096, 1024, 128
SQ = S // 2  # queries per core
N_CORES = 8

TWO_PI = 2.0 * math.pi
INV_TWO_PI = 1.0 / TWO_PI
MAGIC = 12582912.0  # 1.5 * 2**23: fp32 (u + M) - M == round(u)
PI_CLAMP = 3.1415925  # just inside fp32 pi; ACT Sin domain is [-pi, pi]
INV_SQRT_H = 1.0 / math.sqrt(H)


def _cody_waite_consts():
    # Split 2*pi into c1 + c2 + c3, c1/c2 with zeroed low mantissa bits so
    # theta - k*c1 - k*c2 - k*c3 cancels exactly for small integer k.
    def chop(v):
        f = np.float32(v)
        i = f.view(np.uint32) & np.uint32(0xFFFFF000)
        return float(i.view(np.float32))

    c1 = chop(TWO_PI)
    c2 = chop(TWO_PI - c1)
    c3 = float(np.float32(TWO_PI - c1 - c2))
    return c1, c2, c3


C1, C2, C3 = _cody_waite_consts()

_CACHED = None


def _build():
    nc = bacc.Bacc("TRN2", target_bir_lowering=False, debug=False,
                   num_devices=N_CORES)

    x16 = nc.dram_tensor("x16", (S, D), F16, kind="ExternalInput")
    Wq16 = nc.dram_tensor("Wq16", (H, D), F16, kind="ExternalInput")
    Wk16 = nc.dram_tensor("Wk16", (H, D), F16, kind="ExternalInput")
    Wv16 = nc.dram_tensor("Wv16", (H, D), F16, kind="ExternalInput")
    Wo16 = nc.dram_tensor("Wo16", (D, H), F16, kind="ExternalInput")
    bq = nc.dram_tensor("bq", (H, 1), F32, kind="ExternalInput")
    bk = nc.dram_tensor("bk", (H, 1), F32, kind="ExternalInput")
    bv = nc.dram_tensor("bv", (H, 1), F32, kind="ExternalInput")
    bo = nc.dram_tensor("bo", (1, D), F32, kind="ExternalInput")
    wav = nc.dram_tensor("wavelengths", (H, 1), F32, kind="ExternalInput")
    phase = nc.dram_tensor("phase_bias", (H, 1), F32, kind="ExternalInput")
    y = nc.dram_tensor("y", (SQ, D), F32, kind="ExternalOutput")

    with tile.TileContext(nc) as tc, \
            tc.tile_pool(name="const", bufs=1) as const, \
            tc.tile_pool(name="big", bufs=1) as big, \
            tc.tile_pool(name="xa", bufs=2) as xa_pool, \
            tc.tile_pool(name="tmp", bufs=2) as tmp, \
            tc.tile_pool(name="psum_t", bufs=1, space="PSUM") as psum_t, \
            tc.tile_pool(name="psum_mm", bufs=2, space="PSUM") as psum_mm, \
            tc.tile_pool(name="psum_o", bufs=4, space="PSUM") as psum_o:

        # ---- constants -------------------------------------------------
        ident_h = const.tile([128, 128], F16)
        make_identity(nc, ident_h)

        wav_sb = const.tile([H, 1], F32)
        nc.gpsimd.dma_start(wav_sb, wav.ap())
        phase_sb = const.tile([H, 1], F32)
        nc.gpsimd.dma_start(phase_sb, phase.ap())
        bq_sb = const.tile([H, 1], F32)
        nc.gpsimd.dma_start(bq_sb, bq.ap())
        bk_sb = const.tile([H, 1], F32)
        nc.gpsimd.dma_start(bk_sb, bk.ap())
        bv_sb = const.tile([H, 1], F32)
        nc.gpsimd.dma_start(bv_sb, bv.ap())

        inv_w = const.tile([H, 1], F32)
        tw = const.tile([H, 1], F32)
        nc.vector.tensor_scalar(tw, wav_sb, 1e-8, None, mybir.AluOpType.add)
        nc.vector.reciprocal(inv_w, tw)
        cadd_q = const.tile([H, 1], F32)
        nc.vector.tensor_scalar(cadd_q, bq_sb, inv_w, phase_sb,
                                mybir.AluOpType.mult, mybir.AluOpType.add)
        cadd_k = const.tile([H, 1], F32)
        nc.vector.tensor_scalar(cadd_k, bk_sb, inv_w, phase_sb,
                                mybir.AluOpType.mult, mybir.AluOpType.add)
        inv_w2 = const.tile([H, 1], F32)
        nc.vector.tensor_scalar(inv_w2, inv_w, INV_TWO_PI, None,
                                mybir.AluOpType.mult)
        cadd_q2 = const.tile([H, 1], F32)
        nc.vector.tensor_scalar(cadd_q2, cadd_q, INV_TWO_PI, None,
                                mybir.AluOpType.mult)
        cadd_k2 = const.tile([H, 1], F32)
        nc.vector.tensor_scalar(cadd_k2, cadd_k, INV_TWO_PI, None,
                                mybir.AluOpType.mult)

        neg1 = const.tile([128, 1], F32)
        nc.vector.memset(neg1, -1.0)

        bo_row = const.tile([1, D], F32)
        nc.gpsimd.dma_start(bo_row, bo.ap())
        bo_tile = const.tile([128, D], F32)
        nc.gpsimd.partition_broadcast(bo_tile, bo_row)

        # ---- weight transposes (DMA xbar) -----------------------------
        WkT = const.tile([128, 8, 128], F16)
        nc.sync.dma_start_transpose(WkT, Wk16.ap())
        WvT = const.tile([128, 8, 128], F16)
        nc.sync.dma_start_transpose(WvT, Wv16.ap())
        WqT = const.tile([128, 8, 128], F16)
        nc.sync.dma_start_transpose(WqT, Wq16.ap())
        WoT = const.tile([128, D], F16)  # [h, d] (first used in phase C)

        # ---- persistent activations -----------------------------------
        Fq_cos = big.tile([128, SQ], F16)
        Fq_sin = big.tile([128, SQ], F16)
        Fk_cos = big.tile([128, S], F16)
        Fk_sin = big.tile([128, S], F16)
        Vn = big.tile([128, 32, 129], F16)  # [k_part, k_tile, h | ones]
        nc.vector.memset(Vn[:, :, 128:129], 1.0)
        osb = big.tile([128, 16, 129], F32)  # raw [O | denom] per q-subtile

        # ---- phase A: x.T (DMA), projections, sin/cos, V --------------
        def theta_path(pp, cadd, cadd2, cos_slice, sin_slice):
            th = tmp.tile([128, 512], F32, tag="th")
            nc.vector.tensor_scalar(th, pp, inv_w, cadd,
                                    mybir.AluOpType.mult, mybir.AluOpType.add)
            u = tmp.tile([128, 512], F32, tag="u")
            nc.vector.tensor_scalar(u, pp, inv_w2, cadd2,
                                    mybir.AluOpType.mult, mybir.AluOpType.add)
            kk = tmp.tile([128, 512], F32, tag="kk")
            nc.vector.tensor_scalar(kk, u, MAGIC, MAGIC,
                                    mybir.AluOpType.add,
                                    mybir.AluOpType.subtract)
            thr = tmp.tile([128, 512], F32, tag="thr")
            nc.vector.cody_waite_cascade(thr, th, kk, C1, C2, C3)
            nc.vector.tensor_scalar(thr, thr, PI_CLAMP, -PI_CLAMP,
                                    mybir.AluOpType.min, mybir.AluOpType.max)
            nc.scalar.activation(sin_slice, thr, AF.Sin)
            thc = tmp.tile([128, 512], F32, tag="thc")
            nc.vector.add_range_wrap(thc, thr, math.pi / 2, math.pi, TWO_PI)
            nc.scalar.activation(cos_slice, thc, AF.Sin)

        for sc in range(8):
            xt = xa_pool.tile([128, 8, 512], F16, tag="xt", bufs=4)
            nc.sync.dma_start_transpose(
                xt, x16.ap()[sc * 512:(sc + 1) * 512, :])
            if sc == 7:
                nc.sync.dma_start_transpose(WoT, Wo16.ap())

            def proj(wt):
                pp = psum_mm.tile([128, 512], F32, tag="mm512")
                for dc in range(8):
                    nc.tensor.matmul(pp, wt[:, dc, :], xt[:, dc, :],
                                     start=(dc == 0), stop=(dc == 7))
                return pp

            sl = slice(sc * 512, (sc + 1) * 512)
            theta_path(proj(WkT), cadd_k, cadd_k2,
                       Fk_cos[:, sl], Fk_sin[:, sl])

            ppv = proj(WvT)
            v16 = tmp.tile([128, 512], F16, tag="v16")
            nc.vector.tensor_scalar(v16, ppv, bv_sb, None, mybir.AluOpType.add)
            pv = psum_t.tile([128, 512], F16, tag="pt")
            for a in range(4):
                nc.tensor.transpose(pv[:, a * 128:(a + 1) * 128],
                                    v16[:, a * 128:(a + 1) * 128], ident_h)
            nc.vector.tensor_copy(
                Vn[:, sc * 4:(sc + 1) * 4, 0:128],
                pv.rearrange("p (a h) -> p a h", a=4))

            if sc < 4:
                theta_path(proj(WqT), cadd_q, cadd_q2,
                           Fq_cos[:, sl], Fq_sin[:, sl])

        # ---- phase B: attention per 512-query chunk -------------------
        for qc in range(4):
            qsl = slice(qc * 512, (qc + 1) * 512)
            opsums = [psum_o.tile([128, 129], F32, tag="opsum",
                                  name=f"opsum_{qc}_{i}")
                      for i in range(4)]
            for kt in range(32):
                st = psum_mm.tile([128, 512], F32, tag="mm512")
                ksl = slice(kt * 128, (kt + 1) * 128)
                nc.tensor.matmul(st, Fk_cos[:, ksl], Fq_cos[:, qsl],
                                 start=True, stop=False)
                nc.tensor.matmul(st, Fk_sin[:, ksl], Fq_sin[:, qsl],
                                 start=False, stop=True)
                et = tmp.tile([128, 512], F16, tag="et", bufs=3)
                nc.scalar.activation(et, st, AF.Exp, bias=neg1,
                                     scale=INV_SQRT_H)
                for qs in range(4):
                    nc.tensor.matmul(opsums[qs],
                                     et[:, qs * 128:(qs + 1) * 128],
                                     Vn[:, kt, :],
                                     start=(kt == 0), stop=(kt == 31),
                                     skip_group_check=True)
            for qs in range(4):
                nc.vector.tensor_copy(osb[:, qc * 4 + qs, :], opsums[qs])

        # ---- phase C: normalize + output projection -------------------
        for qc in range(4):
            otp = psum_t.tile([128, 512], F16, tag="pt")
            for qs in range(4):
                i = qc * 4 + qs
                rec = tmp.tile([128, 1], F32, tag="rec")
                nc.vector.reciprocal(rec, osb[:, i, 128:129])
                onrm = tmp.tile([128, 128], F16, tag="onrm")
                nc.scalar.activation(onrm, osb[:, i, 0:128], AF.Copy,
                                     scale=rec)
                nc.tensor.transpose(otp[:, qs * 128:(qs + 1) * 128],
                                    onrm, ident_h)
            ot = tmp.tile([128, 512], F16, tag="ot")
            nc.vector.tensor_copy(ot, otp)
            for qs in range(4):
                row = (qc * 4 + qs) * 128
                for half in range(2):
                    yp = psum_t.tile([128, 512], F32, tag="yp")
                    nc.tensor.matmul(yp, ot[:, qs * 128:(qs + 1) * 128],
                                     WoT[:, half * 512:(half + 1) * 512],
                                     start=True, stop=True)
                    ysb = tmp.tile([128, 512], F32, tag="ysb", bufs=3)
                    nc.vector.tensor_add(
                        ysb, yp, bo_tile[:, half * 512:(half + 1) * 512])
                    nc.gpsimd.dma_start(
                        y.ap()[row:row + 128,
                               half * 512:(half + 1) * 512], ysb)

    nc.compile()
    return nc


def get_nc():
    global _CACHED
    if _CACHED is None:
        _CACHED = _build()
    return _CACHED


def _in_maps(inputs):
    x = np.asarray(inputs["x"], np.float32)
    small = {
        "Wq16": np.asarray(inputs["Wq"], np.float16),
        "Wk16": np.asarray(inputs["Wk"], np.float16),
        "Wv16": np.asarray(inputs["Wv"], np.float16),
        "Wo16": np.asarray(inputs["Wo"], np.float16),
        "bq": np.asarray(inputs["bq"], np.float32).reshape(H, 1),
        "bk": np.asarray(inputs["bk"], np.float32).reshape(H, 1),
        "bv": np.asarray(inputs["bv"], np.float32).reshape(H, 1),
        "bo": np.asarray(inputs["bo"], np.float32).reshape(1, D),
        "wavelengths":
            np.asarray(inputs["wavelengths"], np.float32).reshape(H, 1),
        "phase_bias":
            np.asarray(inputs["phase_bias"], np.float32).reshape(H, 1),
    }
    maps = []
    for c in range(N_CORES):
        b, qoff = c // 2, (c % 2) * SQ
        xc = np.roll(x[b], -qoff, axis=0) if qoff else x[b]
        maps.append({"x16": np.ascontiguousarray(xc.astype(np.float16)),
                     **small})
    return maps


def kernel(**inputs):
    from concourse.bass_utils import run_bass_kernel_spmd

    nc = get_nc()
    res = run_bass_kernel_spmd(nc, _in_maps(inputs),
                               core_ids=list(range(N_CORES)))
    out = np.empty((B, S, D), np.float32)
    for c in range(N_CORES):
        b, qoff = c // 2, (c % 2) * SQ
        out[b, qoff:qoff + SQ] = res.results[c]["y"]
    return out
